# revision 6
# baseline (speedup 1.0000x reference)
"""Batched NNLS kernel for Trainium2 (8 NeuronCores, SPMD over columns).

Problem: S = argmin_{s>=0} ||X - A s||^2 column-wise.
  X [256, 2048] f32, A [256, 32] f32  ->  S [32, 2048] f32.

Algorithm (per core, 256 columns, all fixed-iteration / lockstep):
  1. AtA = A^T A [32,32], AtX = A^T X stacked as [128,64] (4 blocks of 32
     coords x 64 columns across the partition dim).
  2. R ~= AtA^{-1} via order-3 Newton-Schulz hyperpower (X0 = (2/L) I,
     L = max row sum of AtA >= lambda_max).
  3. Block principal pivoting with inexact masked solves:
       z0 = R @ AtX, P = (z0 > 0)
       repeat: solve (P AtA P) z = P AtX by PCG preconditioned with
       P R P (few iterations, warm-started), then flip infeasible
       coordinates: P <- (P & z > -eps) | (~P & w > eps),
       w = AtX - AtA relu(z).
     Final refinement solve on the settled mask, output relu(z).

The masked matvec/preconditioner are shared matmuls (blockdiag(AtA),
blockdiag(R) on the tensor engine) + elementwise masking, so the whole
thing is lockstep across 2048 columns with no data-dependent control
flow.
"""

import numpy as np

import concourse.bass as bass
import concourse.mybir as mybir
from concourse import tile

F32 = mybir.dt.float32
AF = mybir.ActivationFunctionType
OP = mybir.AluOpType

M, K, N = 256, 32, 2048
NCORES = 8
NPC = N // NCORES          # columns per core (256)
B = 4                      # partition blocks
W = NPC // B               # columns per block (64)
P128 = 128

GUARD = 1e-25              # reciprocal guard (avoids 0*inf -> NaN)
EPS = 1e-6                 # pivot tolerances (matches reference TOL)

# PCG iterations per BPP round; len(SCHEDULE) mask updates happen between
# rounds, then FINAL_ITERS more refine on the settled mask.
SCHEDULE = (4, 3, 3, 3)
FINAL_ITERS = 3
NS_ITERS = 7               # order-3 Newton-Schulz iterations


def _build_program(schedule=SCHEDULE, final_iters=FINAL_ITERS, ns_iters=NS_ITERS):
    nc = bass.Bass()

    x_d = nc.declare_dram_parameter("x", [M, NPC], F32, isOutput=False)
    a_d = nc.declare_dram_parameter("a", [M, K], F32, isOutput=False)
    eye_d = nc.declare_dram_parameter("eye32", [K, K], F32, isOutput=False)
    bones_d = nc.declare_dram_parameter("bones", [P128, B], F32, isOutput=False)
    bcast_d = nc.declare_dram_parameter("bcast", [B, P128], F32, isOutput=False)
    ones_d = nc.declare_dram_parameter("onesrow", [1, K], F32, isOutput=False)
    s_d = nc.declare_dram_parameter("s", [K, NPC], F32, isOutput=True)

    with tile.TileContext(nc) as tc:
        with (
            tc.tile_pool(name="const", bufs=1) as constp,
            tc.tile_pool(name="state", bufs=1) as statep,
            tc.tile_pool(name="ns", bufs=2) as nsp,
            tc.tile_pool(name="work", bufs=2) as workp,
            tc.tile_pool(name="ps_big", bufs=2, space="PSUM") as ps_big,
            tc.tile_pool(name="ps_dot", bufs=2, space="PSUM") as ps_dot,
            tc.tile_pool(name="ps_sm", bufs=2, space="PSUM") as ps_sm,
        ):
            # ---- loads ----
            eye = constp.tile([K, K], F32, tag="eye")
            bones = constp.tile([P128, B], F32, tag="bones")
            bcast = constp.tile([B, P128], F32, tag="bcast")
            onesr = constp.tile([1, K], F32, tag="onesr")
            a_sb = constp.tile([P128, 2 * K], F32, tag="a_sb")
            x_sb = constp.tile([P128, 2 * NPC], F32, tag="x_sb")

            nc.sync.dma_start(eye[:], eye_d[:])
            nc.sync.dma_start(bones[:], bones_d[:])
            nc.sync.dma_start(bcast[:], bcast_d[:])
            nc.sync.dma_start(onesr[:], ones_d[:])
            # a_sb[p, 32c+k] = A[128c+p, k]
            nc.sync.dma_start(a_sb[:, 0:K], a_d[0:P128, :])
            nc.sync.dma_start(a_sb[:, K:2 * K], a_d[P128:M, :])
            # x_sb[p, 256c+j] = X[128c+p, j]
            nc.sync.dma_start(x_sb[:, 0:NPC], x_d[0:P128, :])
            nc.sync.dma_start(x_sb[:, NPC:2 * NPC], x_d[P128:M, :])

            # ---- AtA [32,32] ----
            ata_ps = ps_sm.tile([K, K], F32, tag="sm")
            nc.tensor.matmul(ata_ps[:], a_sb[:, 0:K], a_sb[:, 0:K],
                             start=True, stop=False)
            nc.tensor.matmul(ata_ps[:], a_sb[:, K:2 * K], a_sb[:, K:2 * K],
                             start=False, stop=True)
            ata = statep.tile([K, K], F32, tag="ata")
            nc.scalar.activation(ata[:], ata_ps[:], AF.Copy)

            # ---- AtX stacked [128, 64]: band b holds AtX[:, 64b:64b+64] ----
            atx_ps = ps_big.tile([P128, W], F32, tag="big")
            for b in range(B):
                nc.tensor.matmul(
                    atx_ps[b * K:(b + 1) * K, :], a_sb[:, 0:K],
                    x_sb[:, b * W:(b + 1) * W], start=True, stop=False,
                    tile_position=(0, b * K))
            for b in range(B):
                nc.tensor.matmul(
                    atx_ps[b * K:(b + 1) * K, :], a_sb[:, K:2 * K],
                    x_sb[:, NPC + b * W:NPC + (b + 1) * W], start=False, stop=True,
                    tile_position=(0, b * K))
            atx = statep.tile([P128, W], F32, tag="atx")
            nc.vector.tensor_copy(atx[:], atx_ps[:])

            # ---- L = max row sum of AtA; X0 = (2/L) I ----
            rs = workp.tile([K, 1], F32, tag="rs")
            nc.vector.tensor_reduce(rs[:], ata[:], mybir.AxisListType.X, OP.add)
            rsT_ps = ps_sm.tile([K, K], F32, tag="sm")
            nc.tensor.transpose(rsT_ps[:1, :K], rs[:], eye[:])
            lmax = workp.tile([1, 1], F32, tag="lmax")
            nc.vector.tensor_reduce(lmax[:], rsT_ps[:1, :K],
                                    mybir.AxisListType.X, OP.max)
            linv = workp.tile([1, 1], F32, tag="linv")
            nc.vector.reciprocal(linv[:], lmax[:])
            c2 = workp.tile([1, 1], F32, tag="c2")
            nc.vector.tensor_scalar_mul(c2[:], linv[:], 2.0)
            cb_ps = ps_sm.tile([K, 1], F32, tag="sm")
            nc.tensor.matmul(cb_ps[:K, :1], onesr[:], c2[:])
            cb = workp.tile([K, 1], F32, tag="cb")
            nc.scalar.activation(cb[:], cb_ps[:K, :1], AF.Copy)
            xi = nsp.tile([K, K], F32, tag="xi")
            nc.vector.tensor_scalar(xi[:], eye[:], cb[:], None, op0=OP.mult)

            # ---- Newton-Schulz order 3: X <- X (I + E (I + E)), E = I - AtA X
            for t in range(ns_iters):
                y_ps = ps_sm.tile([K, K], F32, tag="sm")
                nc.tensor.matmul(y_ps[:], ata[:], xi[:])
                e_sb = nsp.tile([K, K], F32, tag="e")
                nc.vector.tensor_tensor(e_sb[:], eye[:], y_ps[:], OP.subtract)
                e2_ps = ps_sm.tile([K, K], F32, tag="sm")
                nc.tensor.matmul(e2_ps[:], e_sb[:], e_sb[:])
                f1 = nsp.tile([K, K], F32, tag="f1")
                nc.vector.tensor_tensor(f1[:], e_sb[:], e2_ps[:], OP.add)
                f2 = nsp.tile([K, K], F32, tag="f2")
                nc.vector.tensor_tensor(f2[:], eye[:], f1[:], OP.add)
                xn_ps = ps_sm.tile([K, K], F32, tag="sm")
                nc.tensor.matmul(xn_ps[:], xi[:], f2[:])
                xi = nsp.tile([K, K], F32, tag="xi")
                nc.scalar.activation(xi[:], xn_ps[:], AF.Copy)

            # ---- blockdiag(AtA), blockdiag(R) [128,128] ----
            bd_ata = statep.tile([P128, P128], F32, tag="bd_ata")
            bd_r = statep.tile([P128, P128], F32, tag="bd_r")
            nc.vector.memset(bd_ata[:], 0.0)
            nc.vector.memset(bd_r[:], 0.0)
            bd_ps = ps_big.tile([P128, P128], F32, tag="bdps")
            for b in range(B):
                sl = slice(b * K, (b + 1) * K)
                nc.tensor.matmul(bd_ps[sl, sl], ata[:], eye[:],
                                 tile_position=(0, b * K))
            for b in range(B):
                sl = slice(b * K, (b + 1) * K)
                nc.vector.tensor_copy(bd_ata[sl, sl], bd_ps[sl, sl])
            bd_ps2 = ps_big.tile([P128, P128], F32, tag="bdps")
            for b in range(B):
                sl = slice(b * K, (b + 1) * K)
                nc.tensor.matmul(bd_ps2[sl, sl], xi[:], eye[:],
                                 tile_position=(0, b * K))
            for b in range(B):
                sl = slice(b * K, (b + 1) * K)
                nc.vector.tensor_copy(bd_r[sl, sl], bd_ps2[sl, sl])

            # ---- init: z0 = R AtX (stacked), P = z0 > 0, z = z0 * P ----
            z0_ps = ps_big.tile([P128, W], F32, tag="big")
            nc.tensor.matmul(z0_ps[:], bd_r[:], atx[:])
            pm = workp.tile([P128, W], F32, tag="pm")
            nc.vector.tensor_single_scalar(pm[:], z0_ps[:], 0.0, OP.is_gt)
            z = statep.tile([P128, W], F32, tag="z")
            nc.vector.tensor_tensor(z[:], z0_ps[:], pm[:], OP.mult)

            rr = statep.tile([P128, W], F32, tag="rr")
            dd = statep.tile([P128, W], F32, tag="dd")
            qq = statep.tile([P128, W], F32, tag="qq")
            ee = statep.tile([P128, W], F32, tag="ee")
            t1 = statep.tile([P128, W], F32, tag="t1")
            t2 = statep.tile([P128, W], F32, tag="t2")

            def cg_solve(pm, n_iters):
                """PCG on the masked system, warm start from z (masked)."""
                # r = P (AtX - AtA z); d = P (R r); rho = sum_k r*d
                g_ps = ps_big.tile([P128, W], F32, tag="big")
                nc.tensor.matmul(g_ps[:], bd_ata[:], z[:])
                nc.vector.tensor_tensor(t1[:], atx[:], g_ps[:], OP.subtract)
                nc.vector.tensor_tensor(rr[:], t1[:], pm[:], OP.mult)
                e_ps = ps_big.tile([P128, W], F32, tag="big")
                nc.tensor.matmul(e_ps[:], bd_r[:], rr[:])
                nc.vector.tensor_tensor(dd[:], e_ps[:], pm[:], OP.mult)
                nc.vector.tensor_tensor(t1[:], rr[:], dd[:], OP.mult)
                rho_ps = ps_dot.tile([B, W], F32, tag="dot")
                nc.tensor.matmul(rho_ps[:], bones[:], t1[:])
                rho = workp.tile([B, W], F32, tag="rho")
                nc.scalar.activation(rho[:], rho_ps[:], AF.Copy, bias=GUARD)

                for it in range(n_iters):
                    last = it == n_iters - 1
                    # q = P (AtA d)
                    q_ps = ps_big.tile([P128, W], F32, tag="big")
                    nc.tensor.matmul(q_ps[:], bd_ata[:], dd[:])
                    nc.vector.tensor_tensor(qq[:], q_ps[:], pm[:], OP.mult)
                    # alpha = rho / (d.q)
                    nc.vector.tensor_tensor(t1[:], dd[:], qq[:], OP.mult)
                    dq_ps = ps_dot.tile([B, W], F32, tag="dot")
                    nc.tensor.matmul(dq_ps[:], bones[:], t1[:])
                    dq_g = workp.tile([B, W], F32, tag="dq_g")
                    nc.scalar.activation(dq_g[:], dq_ps[:], AF.Copy, bias=GUARD)
                    inv_dq = workp.tile([B, W], F32, tag="inv_dq")
                    nc.vector.reciprocal(inv_dq[:], dq_g[:])
                    alpha = workp.tile([B, W], F32, tag="alpha")
                    nc.vector.tensor_tensor(alpha[:], rho[:], inv_dq[:], OP.mult)
                    abc_ps = ps_big.tile([P128, W], F32, tag="big")
                    nc.tensor.matmul(abc_ps[:], bcast[:], alpha[:])
                    # z += alpha d
                    nc.vector.tensor_tensor(t1[:], abc_ps[:], dd[:], OP.mult)
                    nc.vector.tensor_tensor(z[:], z[:], t1[:], OP.add)
                    if last:
                        break
                    # r -= alpha q
                    nc.vector.tensor_tensor(t2[:], abc_ps[:], qq[:], OP.mult)
                    nc.vector.tensor_tensor(rr[:], rr[:], t2[:], OP.subtract)
                    # e = P (R r); rho' = sum_k r*e; beta = rho'/rho
                    e_ps = ps_big.tile([P128, W], F32, tag="big")
                    nc.tensor.matmul(e_ps[:], bd_r[:], rr[:])
                    nc.vector.tensor_tensor(ee[:], e_ps[:], pm[:], OP.mult)
                    nc.vector.tensor_tensor(t1[:], rr[:], ee[:], OP.mult)
                    rho_new_ps = ps_dot.tile([B, W], F32, tag="dot")
                    nc.tensor.matmul(rho_new_ps[:], bones[:], t1[:])
                    inv_rho = workp.tile([B, W], F32, tag="inv_rho")
                    nc.vector.reciprocal(inv_rho[:], rho[:])
                    rho = workp.tile([B, W], F32, tag="rho")
                    nc.scalar.activation(rho[:], rho_new_ps[:], AF.Copy, bias=GUARD)
                    beta = workp.tile([B, W], F32, tag="beta")
                    nc.vector.tensor_tensor(beta[:], rho[:], inv_rho[:], OP.mult)
                    bbc_ps = ps_big.tile([P128, W], F32, tag="big")
                    nc.tensor.matmul(bbc_ps[:], bcast[:], beta[:])
                    # d = e + beta d
                    nc.vector.tensor_tensor(t1[:], bbc_ps[:], dd[:], OP.mult)
                    nc.vector.tensor_tensor(dd[:], ee[:], t1[:], OP.add)

            for rnd, n_iters in enumerate(schedule):
                cg_solve(pm, n_iters)
                # mask update: P <- (P & z > -eps) | (~P & w > eps)
                st = workp.tile([P128, W], F32, tag="st")
                nc.vector.tensor_scalar_max(st[:], z[:], 0.0)
                w_ps = ps_big.tile([P128, W], F32, tag="big")
                nc.tensor.matmul(w_ps[:], bd_ata[:], st[:])
                wv = workp.tile([P128, W], F32, tag="wv")
                nc.vector.tensor_tensor(wv[:], atx[:], w_ps[:], OP.subtract)
                b_dual = workp.tile([P128, W], F32, tag="b_dual")
                nc.vector.tensor_single_scalar(b_dual[:], wv[:], EPS, OP.is_gt)
                a_pri = workp.tile([P128, W], F32, tag="a_pri")
                nc.vector.tensor_single_scalar(a_pri[:], z[:], -EPS, OP.is_gt)
                # pm_new = b_dual + pm * (a_pri - b_dual)  (select via arith)
                nc.vector.tensor_tensor(a_pri[:], a_pri[:], b_dual[:], OP.subtract)
                pm_new = workp.tile([P128, W], F32, tag="pm")
                nc.vector.tensor_tensor(pm_new[:], pm[:], a_pri[:], OP.mult)
                nc.vector.tensor_tensor(pm_new[:], pm_new[:], b_dual[:], OP.add)
                pm = pm_new
                nc.vector.tensor_tensor(z[:], st[:], pm[:], OP.mult)

            cg_solve(pm, final_iters)

            # ---- output: s = relu(z), unstack [128,64] -> [32,256] via DMA
            out_sb = workp.tile([P128, W], F32, tag="out")
            nc.vector.tensor_scalar_max(out_sb[:], z[:], 0.0)
            for b in range(B):
                nc.sync.dma_start(s_d[:, b * W:(b + 1) * W],
                                  out_sb[b * K:(b + 1) * K, :])

    _split_multi_waits(nc)
    return nc


def _split_multi_waits(nc, max_waits=1):
    """walrus in this toolchain supports one sync-wait per instruction;
    move extra waits onto chained same-engine NOPs ahead of the owner."""
    n = 0
    for fn in nc.m.functions:
        for blk in fn.blocks:
            new_insts = []
            for inst in blk.instructions:
                si = inst.sync_info
                if si is not None and len(si.on_wait) > max_waits:
                    waits = list(si.on_wait)
                    si.on_wait = waits[:max_waits]
                    waits = waits[max_waits:]
                    while waits:
                        chunk, waits = waits[:max_waits], waits[max_waits:]
                        nop = mybir.InstNoOp(
                            name=f"I-waitsplit-{nc.next_id()}", ins=[], outs=[])
                        nop.engine = inst.engine
                        nop.sync_info = mybir.SyncInfo(on_wait=chunk, on_update=[])
                        nc.register_instruction(nop)
                        new_insts.append(nop)
                        n += 1
                new_insts.append(inst)
            blk.instructions[:] = new_insts
    return n


def _consts():
    eye = np.eye(K, dtype=np.float32)
    bones = np.zeros((P128, B), dtype=np.float32)
    for b in range(B):
        bones[b * K:(b + 1) * K, b] = 1.0
    bcast = bones.T.copy()
    onesrow = np.ones((1, K), dtype=np.float32)
    return eye, bones, bcast, onesrow


_CACHED = {}


def kernel(input, A):
    X = np.ascontiguousarray(np.asarray(input, dtype=np.float32))
    A = np.ascontiguousarray(np.asarray(A, dtype=np.float32))
    assert X.shape == (M, N) and A.shape == (M, K)

    from concourse.bass_utils import run_bass_kernel_spmd

    if "nc" not in _CACHED:
        _CACHED["nc"] = _build_program()
    nc = _CACHED["nc"]

    eye, bones, bcast, onesrow = _consts()
    in_maps = []
    for c in range(NCORES):
        in_maps.append({
            "x": np.ascontiguousarray(X[:, c * NPC:(c + 1) * NPC]),
            "a": A,
            "eye32": eye,
            "bones": bones,
            "bcast": bcast,
            "onesrow": onesrow,
        })
    res = run_bass_kernel_spmd(nc, in_maps, list(range(NCORES)))
    out = np.concatenate([res.results[c]["s"] for c in range(NCORES)], axis=1)
    return out.astype(np.float32)


# revision 8
# speedup vs baseline: 1.2988x; 1.2988x over previous
"""Batched NNLS kernel for Trainium2 (8 NeuronCores, SPMD over columns).

Problem: S = argmin_{s>=0} ||X - A s||^2 column-wise.
  X [256, 2048] f32, A [256, 32] f32  ->  S [32, 2048] f32.

Algorithm (per core, 256 columns, all fixed-iteration / lockstep):
  1. AtA = A^T A [32,32], AtX = A^T X stacked as [128,64] (4 blocks of 32
     coords x 64 columns across the partition dim).
  2. R ~= AtA^{-1} via order-3 Newton-Schulz hyperpower in bf16
     (X0 = (2/L) I, L = max row sum of AtA >= lambda_max).
  3. Block principal pivoting with inexact masked solves:
       z0 = R @ AtX, P = (z0 > 0)
       repeat: solve (P AtA P) z = P AtX by PCG preconditioned with
       P R P (few iterations, warm-started), then flip infeasible
       coordinates: P <- (P & z > -eps) | (~P & w > eps),
       w = AtX - AtA relu(z).
     Final refinement solve on the settled mask, output relu(z).

Mixed precision: the per-iteration CG matvecs/dots/coefficients run in
bf16 (1-pass PE matmuls); correctness comes from fp32 state (z, r) and
the fp32 residual restart at every mask round (iterative-refinement
structure), plus fp32 mask-update matvecs. Validated in emulation to
~2e-5 relative absmax worst-case across seeds.
"""

import numpy as np

import concourse.bass as bass
import concourse.mybir as mybir
from concourse import tile

F32 = mybir.dt.float32
BF16 = mybir.dt.bfloat16
AF = mybir.ActivationFunctionType
OP = mybir.AluOpType

M, K, N = 256, 32, 2048
NCORES = 8
NPC = N // NCORES          # columns per core (256)
B = 4                      # partition blocks
W = NPC // B               # columns per block (64)
P128 = 128

GUARD = 1e-25              # reciprocal guard (avoids 0*inf -> NaN)
EPS = 1e-6                 # pivot tolerances (matches reference TOL)

SCHEDULE = (3, 3, 3, 3)    # PCG iterations per BPP round
FINAL_ITERS = 3            # refinement iterations on the settled mask
NS_ITERS = 6               # order-3 Newton-Schulz iterations (bf16)


def _build_program(schedule=SCHEDULE, final_iters=FINAL_ITERS, ns_iters=NS_ITERS):
    nc = bass.Bass()

    x_d = nc.declare_dram_parameter("x", [M, NPC], F32, isOutput=False)
    a_d = nc.declare_dram_parameter("a", [M, K], F32, isOutput=False)
    eye_d = nc.declare_dram_parameter("eye32", [K, K], F32, isOutput=False)
    bones_d = nc.declare_dram_parameter("bones", [P128, B], F32, isOutput=False)
    bcast_d = nc.declare_dram_parameter("bcast", [B, P128], F32, isOutput=False)
    ones_d = nc.declare_dram_parameter("onesrow", [1, K], F32, isOutput=False)
    s_d = nc.declare_dram_parameter("s", [K, NPC], F32, isOutput=True)

    with tile.TileContext(nc) as tc:
        with (
            tc.tile_pool(name="const", bufs=1) as constp,
            tc.tile_pool(name="state", bufs=1) as statep,
            tc.tile_pool(name="ns", bufs=2) as nsp,
            tc.tile_pool(name="work", bufs=2) as workp,
            tc.tile_pool(name="ps_big", bufs=2, space="PSUM") as ps_big,
            tc.tile_pool(name="ps_dot", bufs=2, space="PSUM") as ps_dot,
            tc.tile_pool(name="ps_sm", bufs=2, space="PSUM") as ps_sm,
        ):
            # ---- loads ----
            eye = constp.tile([K, K], F32, tag="eye")
            eye_bf = constp.tile([K, K], BF16, tag="eye_bf")
            bones_bf = constp.tile([P128, B], BF16, tag="bones")
            bcast_bf = constp.tile([B, P128], BF16, tag="bcast")
            onesr = constp.tile([1, K], F32, tag="onesr")
            a_sb = constp.tile([P128, 2 * K], F32, tag="a_sb")
            x_sb = constp.tile([P128, 2 * NPC], F32, tag="x_sb")

            nc.sync.dma_start(eye[:], eye_d[:])
            bones_f = workp.tile([P128, B], F32, tag="bones_f")
            bcast_f = workp.tile([B, P128], F32, tag="bcast_f")
            nc.sync.dma_start(bones_f[:], bones_d[:])
            nc.sync.dma_start(bcast_f[:], bcast_d[:])
            nc.sync.dma_start(onesr[:], ones_d[:])
            nc.vector.tensor_copy(eye_bf[:], eye[:])
            nc.vector.tensor_copy(bones_bf[:], bones_f[:])
            nc.vector.tensor_copy(bcast_bf[:], bcast_f[:])
            # a_sb[p, 32c+k] = A[128c+p, k]
            nc.sync.dma_start(a_sb[:, 0:K], a_d[0:P128, :])
            nc.sync.dma_start(a_sb[:, K:2 * K], a_d[P128:M, :])
            # x_sb[p, 256c+j] = X[128c+p, j]
            nc.sync.dma_start(x_sb[:, 0:NPC], x_d[0:P128, :])
            nc.sync.dma_start(x_sb[:, NPC:2 * NPC], x_d[P128:M, :])

            # ---- AtA [32,32] (fp32) ----
            ata_ps = ps_sm.tile([K, K], F32, tag="sm")
            nc.tensor.matmul(ata_ps[:], a_sb[:, 0:K], a_sb[:, 0:K],
                             start=True, stop=False)
            nc.tensor.matmul(ata_ps[:], a_sb[:, K:2 * K], a_sb[:, K:2 * K],
                             start=False, stop=True)
            ata = statep.tile([K, K], F32, tag="ata")
            nc.scalar.activation(ata[:], ata_ps[:], AF.Copy)
            ata_bf = statep.tile([K, K], BF16, tag="ata_bf")
            nc.vector.tensor_copy(ata_bf[:], ata_ps[:])

            # ---- AtX stacked [128, 64]: band b holds AtX[:, 64b:64b+64] ----
            atx_ps = ps_big.tile([P128, W], F32, tag="big")
            for b in range(B):
                nc.tensor.matmul(
                    atx_ps[b * K:(b + 1) * K, :], a_sb[:, 0:K],
                    x_sb[:, b * W:(b + 1) * W], start=True, stop=False,
                    tile_position=(0, b * K))
            for b in range(B):
                nc.tensor.matmul(
                    atx_ps[b * K:(b + 1) * K, :], a_sb[:, K:2 * K],
                    x_sb[:, NPC + b * W:NPC + (b + 1) * W], start=False, stop=True,
                    tile_position=(0, b * K))
            atx = statep.tile([P128, W], F32, tag="atx")
            nc.vector.tensor_copy(atx[:], atx_ps[:])
            atx_bf = statep.tile([P128, W], BF16, tag="atx_bf")
            nc.scalar.activation(atx_bf[:], atx_ps[:], AF.Copy)

            # ---- L = max row sum of AtA; X0 = (2/L) I (bf16) ----
            rs = workp.tile([K, 1], F32, tag="rs")
            nc.vector.tensor_reduce(rs[:], ata[:], mybir.AxisListType.X, OP.add)
            rsT_ps = ps_sm.tile([K, K], F32, tag="sm")
            nc.tensor.transpose(rsT_ps[:1, :K], rs[:], eye[:])
            lmax = workp.tile([1, 1], F32, tag="lmax")
            nc.vector.tensor_reduce(lmax[:], rsT_ps[:1, :K],
                                    mybir.AxisListType.X, OP.max)
            linv = workp.tile([1, 1], F32, tag="linv")
            nc.vector.reciprocal(linv[:], lmax[:])
            c2 = workp.tile([1, 1], F32, tag="c2")
            nc.vector.tensor_scalar_mul(c2[:], linv[:], 2.0)
            cb_ps = ps_sm.tile([K, 1], F32, tag="sm")
            nc.tensor.matmul(cb_ps[:K, :1], onesr[:], c2[:])
            cb = workp.tile([K, 1], F32, tag="cb")
            nc.scalar.activation(cb[:], cb_ps[:K, :1], AF.Copy)
            xi = nsp.tile([K, K], BF16, tag="xi")
            nc.vector.tensor_scalar(xi[:], eye[:], cb[:], None, op0=OP.mult)

            # ---- Newton-Schulz order 3 (bf16): X <- X (I + E (I + E)) ----
            for t in range(ns_iters):
                y_ps = ps_sm.tile([K, K], F32, tag="sm")
                nc.tensor.matmul(y_ps[:], ata_bf[:], xi[:])
                e_sb = nsp.tile([K, K], BF16, tag="e")
                nc.vector.tensor_tensor(e_sb[:], eye[:], y_ps[:], OP.subtract)
                e2_ps = ps_sm.tile([K, K], F32, tag="sm")
                nc.tensor.matmul(e2_ps[:], e_sb[:], e_sb[:])
                f1 = nsp.tile([K, K], BF16, tag="f1")
                nc.vector.tensor_tensor(f1[:], e_sb[:], e2_ps[:], OP.add)
                f2 = nsp.tile([K, K], BF16, tag="f2")
                nc.vector.tensor_tensor(f2[:], eye[:], f1[:], OP.add)
                xn_ps = ps_sm.tile([K, K], F32, tag="sm")
                nc.tensor.matmul(xn_ps[:], xi[:], f2[:])
                xi = nsp.tile([K, K], BF16, tag="xi")
                nc.scalar.activation(xi[:], xn_ps[:], AF.Copy)

            # ---- blockdiag(AtA) fp32 + bf16, blockdiag(R) bf16 [128,128] ----
            bd_ata = statep.tile([P128, P128], F32, tag="bd_ata")
            bd_ata_bf = statep.tile([P128, P128], BF16, tag="bd_ata_bf")
            bd_r = statep.tile([P128, P128], BF16, tag="bd_r")
            nc.vector.memset(bd_ata[:], 0.0)
            nc.vector.memset(bd_ata_bf[:], 0.0)
            nc.vector.memset(bd_r[:], 0.0)
            bd_ps = ps_big.tile([P128, P128], F32, tag="bdps")
            for b in range(B):
                sl = slice(b * K, (b + 1) * K)
                nc.tensor.matmul(bd_ps[sl, sl], ata[:], eye[:],
                                 tile_position=(0, b * K))
            for b in range(B):
                sl = slice(b * K, (b + 1) * K)
                nc.vector.tensor_copy(bd_ata[sl, sl], bd_ps[sl, sl])
                nc.scalar.activation(bd_ata_bf[sl, sl], bd_ps[sl, sl], AF.Copy)
            bd_ps2 = ps_big.tile([P128, P128], F32, tag="bdps")
            for b in range(B):
                sl = slice(b * K, (b + 1) * K)
                nc.tensor.matmul(bd_ps2[sl, sl], xi[:], eye_bf[:],
                                 tile_position=(0, b * K))
            for b in range(B):
                sl = slice(b * K, (b + 1) * K)
                nc.vector.tensor_copy(bd_r[sl, sl], bd_ps2[sl, sl])

            # ---- init: z0 = R AtX (bf16 matvec), P = z0 > 0, z = z0 * P ----
            z0_ps = ps_big.tile([P128, W], F32, tag="big")
            nc.tensor.matmul(z0_ps[:], bd_r[:], atx_bf[:])
            pm = workp.tile([P128, W], F32, tag="pm")
            nc.vector.tensor_single_scalar(pm[:], z0_ps[:], 0.0, OP.is_gt)
            z = statep.tile([P128, W], F32, tag="z")
            nc.vector.tensor_tensor(z[:], z0_ps[:], pm[:], OP.mult)

            rr = statep.tile([P128, W], F32, tag="rr")      # fp32 residual
            r_bf = statep.tile([P128, W], BF16, tag="r_bf")
            qq = statep.tile([P128, W], F32, tag="qq")      # fp32 (exact B d)
            dd = statep.tile([P128, W], BF16, tag="dd")     # bf16 direction
            ee = statep.tile([P128, W], BF16, tag="ee")
            t1 = statep.tile([P128, W], F32, tag="t1")
            t2 = statep.tile([P128, W], F32, tag="t2")
            t1b = statep.tile([P128, W], BF16, tag="t1b")

            def cg_solve(pm, n_iters):
                """PCG on the masked system, warm start from z (masked)."""
                # fp32 restart: r = P (AtX - AtA z)
                g_ps = ps_big.tile([P128, W], F32, tag="big")
                nc.tensor.matmul(g_ps[:], bd_ata[:], z[:])
                nc.vector.tensor_tensor(t1[:], atx[:], g_ps[:], OP.subtract)
                nc.vector.tensor_tensor(rr[:], t1[:], pm[:], OP.mult)
                nc.scalar.activation(r_bf[:], rr[:], AF.Copy)
                e_ps = ps_big.tile([P128, W], F32, tag="big")
                nc.tensor.matmul(e_ps[:], bd_r[:], r_bf[:])
                nc.vector.tensor_tensor(dd[:], e_ps[:], pm[:], OP.mult)
                nc.vector.tensor_tensor(t1b[:], rr[:], dd[:], OP.mult)
                rho_ps = ps_dot.tile([B, W], F32, tag="dot")
                nc.tensor.matmul(rho_ps[:], bones_bf[:], t1b[:])
                rho = workp.tile([B, W], F32, tag="rho")
                nc.scalar.activation(rho[:], rho_ps[:], AF.Copy, bias=GUARD)

                for it in range(n_iters):
                    last = it == n_iters - 1
                    # q = P (AtA d)   [bf16 matvec -> fp32 psum]
                    q_ps = ps_big.tile([P128, W], F32, tag="big")
                    nc.tensor.matmul(q_ps[:], bd_ata_bf[:], dd[:])
                    nc.vector.tensor_tensor(qq[:], q_ps[:], pm[:], OP.mult)
                    # alpha = rho / (d.q)
                    nc.vector.tensor_tensor(t1b[:], dd[:], qq[:], OP.mult)
                    dq_ps = ps_dot.tile([B, W], F32, tag="dot")
                    nc.tensor.matmul(dq_ps[:], bones_bf[:], t1b[:])
                    dq_g = workp.tile([B, W], F32, tag="dq_g")
                    nc.scalar.activation(dq_g[:], dq_ps[:], AF.Copy, bias=GUARD)
                    inv_dq = workp.tile([B, W], F32, tag="inv_dq")
                    nc.vector.reciprocal(inv_dq[:], dq_g[:])
                    alpha = workp.tile([B, W], BF16, tag="alpha")
                    nc.vector.tensor_tensor(alpha[:], rho[:], inv_dq[:], OP.mult)
                    abc_ps = ps_big.tile([P128, W], F32, tag="big")
                    nc.tensor.matmul(abc_ps[:], bcast_bf[:], alpha[:])
                    # z += alpha d   (fp32)
                    nc.vector.tensor_tensor(t1[:], abc_ps[:], dd[:], OP.mult)
                    nc.vector.tensor_tensor(z[:], z[:], t1[:], OP.add)
                    if last:
                        break
                    # r -= alpha q   (fp32)
                    nc.vector.tensor_tensor(t2[:], abc_ps[:], qq[:], OP.mult)
                    nc.vector.tensor_tensor(rr[:], rr[:], t2[:], OP.subtract)
                    nc.scalar.activation(r_bf[:], rr[:], AF.Copy)
                    # e = P (R r); rho' = sum_k r*e; beta = rho'/rho
                    e_ps = ps_big.tile([P128, W], F32, tag="big")
                    nc.tensor.matmul(e_ps[:], bd_r[:], r_bf[:])
                    nc.vector.tensor_tensor(ee[:], e_ps[:], pm[:], OP.mult)
                    nc.vector.tensor_tensor(t1b[:], rr[:], ee[:], OP.mult)
                    rho_new_ps = ps_dot.tile([B, W], F32, tag="dot")
                    nc.tensor.matmul(rho_new_ps[:], bones_bf[:], t1b[:])
                    inv_rho = workp.tile([B, W], F32, tag="inv_rho")
                    nc.vector.reciprocal(inv_rho[:], rho[:])
                    rho = workp.tile([B, W], F32, tag="rho")
                    nc.scalar.activation(rho[:], rho_new_ps[:], AF.Copy, bias=GUARD)
                    beta = workp.tile([B, W], BF16, tag="beta")
                    nc.vector.tensor_tensor(beta[:], rho[:], inv_rho[:], OP.mult)
                    bbc_ps = ps_big.tile([P128, W], F32, tag="big")
                    nc.tensor.matmul(bbc_ps[:], bcast_bf[:], beta[:])
                    # d = e + beta d   (bf16)
                    nc.vector.tensor_tensor(t1[:], bbc_ps[:], dd[:], OP.mult)
                    nc.vector.tensor_tensor(dd[:], ee[:], t1[:], OP.add)

            for rnd, n_iters in enumerate(schedule):
                cg_solve(pm, n_iters)
                # mask update: P <- (P & z > -eps) | (~P & w > eps)  [fp32]
                st = workp.tile([P128, W], F32, tag="st")
                nc.vector.tensor_scalar_max(st[:], z[:], 0.0)
                w_ps = ps_big.tile([P128, W], F32, tag="big")
                nc.tensor.matmul(w_ps[:], bd_ata[:], st[:])
                wv = workp.tile([P128, W], F32, tag="wv")
                nc.vector.tensor_tensor(wv[:], atx[:], w_ps[:], OP.subtract)
                b_dual = workp.tile([P128, W], F32, tag="b_dual")
                nc.vector.tensor_single_scalar(b_dual[:], wv[:], EPS, OP.is_gt)
                a_pri = workp.tile([P128, W], F32, tag="a_pri")
                nc.vector.tensor_single_scalar(a_pri[:], z[:], -EPS, OP.is_gt)
                # pm_new = b_dual + pm * (a_pri - b_dual)  (select via arith)
                nc.vector.tensor_tensor(a_pri[:], a_pri[:], b_dual[:], OP.subtract)
                pm_new = workp.tile([P128, W], F32, tag="pm")
                nc.vector.tensor_tensor(pm_new[:], pm[:], a_pri[:], OP.mult)
                nc.vector.tensor_tensor(pm_new[:], pm_new[:], b_dual[:], OP.add)
                pm = pm_new
                nc.vector.tensor_tensor(z[:], st[:], pm[:], OP.mult)

            cg_solve(pm, final_iters)

            # ---- output: s = relu(z), unstack [128,64] -> [32,256] via DMA
            out_sb = workp.tile([P128, W], F32, tag="out")
            nc.vector.tensor_scalar_max(out_sb[:], z[:], 0.0)
            for b in range(B):
                nc.sync.dma_start(s_d[:, b * W:(b + 1) * W],
                                  out_sb[b * K:(b + 1) * K, :])

    _split_multi_waits(nc)
    return nc


def _split_multi_waits(nc, max_waits=1):
    """walrus in this toolchain supports one sync-wait per instruction;
    move extra waits onto chained same-engine NOPs ahead of the owner."""
    n = 0
    for fn in nc.m.functions:
        for blk in fn.blocks:
            new_insts = []
            for inst in blk.instructions:
                si = inst.sync_info
                if si is not None and len(si.on_wait) > max_waits:
                    waits = list(si.on_wait)
                    si.on_wait = waits[:max_waits]
                    waits = waits[max_waits:]
                    while waits:
                        chunk, waits = waits[:max_waits], waits[max_waits:]
                        nop = mybir.InstNoOp(
                            name=f"I-waitsplit-{nc.next_id()}", ins=[], outs=[])
                        nop.engine = inst.engine
                        nop.sync_info = mybir.SyncInfo(on_wait=chunk, on_update=[])
                        nc.register_instruction(nop)
                        new_insts.append(nop)
                        n += 1
                new_insts.append(inst)
            blk.instructions[:] = new_insts
    return n


def _consts():
    eye = np.eye(K, dtype=np.float32)
    bones = np.zeros((P128, B), dtype=np.float32)
    for b in range(B):
        bones[b * K:(b + 1) * K, b] = 1.0
    bcast = bones.T.copy()
    onesrow = np.ones((1, K), dtype=np.float32)
    return eye, bones, bcast, onesrow


_CACHED = {}


def kernel(input, A):
    X = np.ascontiguousarray(np.asarray(input, dtype=np.float32))
    A = np.ascontiguousarray(np.asarray(A, dtype=np.float32))
    assert X.shape == (M, N) and A.shape == (M, K)

    from concourse.bass_utils import run_bass_kernel_spmd

    if "nc" not in _CACHED:
        _CACHED["nc"] = _build_program()
    nc = _CACHED["nc"]

    eye, bones, bcast, onesrow = _consts()
    in_maps = []
    for c in range(NCORES):
        in_maps.append({
            "x": np.ascontiguousarray(X[:, c * NPC:(c + 1) * NPC]),
            "a": A,
            "eye32": eye,
            "bones": bones,
            "bcast": bcast,
            "onesrow": onesrow,
        })
    res = run_bass_kernel_spmd(nc, in_maps, list(range(NCORES)))
    out = np.concatenate([res.results[c]["s"] for c in range(NCORES)], axis=1)
    return out.astype(np.float32)
